# revision 26
# baseline (speedup 1.0000x reference)
"""Bass/Trainium2 kernel for nn_Attn_51127290691658.

Reference computation (S=1024, B=64, H=512):
    cat    = concat([broadcast(hidden), encoder_outputs], -1)   [S,B,2H]
    energy = tanh(cat @ W_attn.T + b_attn)                      [S,B,H]
    scores = energy @ beta                                      [S,B,1]
    out    = softmax(scores.transpose(0,2,1), axis=0)           [S,1,B]

Decomposition used here (W1 = W_attn[:, :H], W2 = W_attn[:, H:]):
    U[b,h]      = W1[h,:] . hidden[b,:] + b_attn[h]        (tiny)
    energyT[h,s] = tanh(W2 @ E_b^T + U[:,b])   per batch b (big)
    score[b,s]  = beta . tanh_energy[:, s]
    out[s,b]    = softmax over s

Sharding: data-parallel over B across 8 cores (8 batch elements/core);
W_attn/b_attn/beta replicated. Softmax is local per batch element.

Data path: fp16 inputs to the PE matmuls (full-rate 1 cyc/row on TRN2),
fp32 PSUM accumulation, fp32 softmax. Measured end-to-end absmax error
vs the fp32 reference ~3.5e-3 (softmax output scale is 1.0).
"""

import sys
import types

import numpy as np

S, B, H = 1024, 64, 512
NCORES = 8
BC = B // NCORES  # 8 batch elements per core
KC = H // 128     # 4 contraction chunks
HC = H // 128     # 4 output h chunks
SGS = 2           # two 512-wide s groups
SG = S // SGS     # 512
SI = S // 128     # 8 s chunks of 128


def _install_axon_hooks_shim():
    """The container image's `antenv` lacks `axon_hooks`; without it,
    run_bass_kernel_spmd(trace=True) cannot find the NTFF hook. Register a
    minimal in-memory module and install the ctypes-based hook if available.
    Harmless when tracing is not requested."""
    try:
        import antenv
    except ImportError:
        return
    if "antenv.axon_hooks" in sys.modules:
        return
    mod = types.ModuleType("antenv.axon_hooks")
    mod._hook = None

    def set_axon_ntff_profile_hook(h):
        mod._hook = h

    def get_axon_ntff_profile_hook():
        return mod._hook

    mod.set_axon_ntff_profile_hook = set_axon_ntff_profile_hook
    mod.get_axon_ntff_profile_hook = get_axon_ntff_profile_hook
    sys.modules["antenv.axon_hooks"] = mod
    antenv.axon_hooks = mod
    try:
        from trn_agent_boot.trn_boot import _ntff_profile_via_ctypes

        hook = _ntff_profile_via_ctypes("/opt/axon/libaxon_pjrt.so")
        if hook is not None:
            set_axon_ntff_profile_hook(hook)
    except Exception:
        pass


_install_axon_hooks_shim()

import os  # noqa: E402

if os.environ.get("BASS_LDW_OPT") == "1":
    import concourse.bass_utils as _bu

    _orig_run_command = _bu.run_command

    def _patched_run_command(argv, **kw):
        argv = [
            a.replace("--enable-ldw-opt=false", "--enable-ldw-opt=true") for a in argv
        ]
        return _orig_run_command(argv, **kw)

    _bu.run_command = _patched_run_command

import concourse.bass as bass  # noqa: E402
import concourse.mybir as mybir  # noqa: E402
import concourse.tile as tile  # noqa: E402
from concourse.bass_utils import run_bass_kernel_spmd  # noqa: E402
from concourse.masks import make_identity  # noqa: E402

F32 = mybir.dt.float32
F16 = mybir.dt.float16


def _split_waits(nc, max_waits=1):
    """The walrus build in this container encodes at most one sem-wait per
    instruction ("Too many sync wait commands" otherwise). Tile emits up to
    ~5. Splitting excess waits into preceding same-engine NoOps is
    semantically identical (engine queues execute in order)."""
    ctr = 0
    for fn in nc.m.functions:
        for blk in fn.blocks:
            insts = list(blk.instructions)
            new = []
            changed = False
            for inst in insts:
                si = inst.sync_info
                if si is not None and len(si.on_wait) > max_waits:
                    waits = list(si.on_wait)
                    keep = waits[-max_waits:]
                    extra = waits[:-max_waits]
                    for i in range(0, len(extra), max_waits):
                        ctr += 1
                        new.append(
                            mybir.InstNoOp(
                                name=f"WSPLIT-{id(nc) & 0xFFFF}-{ctr}",
                                engine=inst.engine,
                                bass_nofuse=True,
                                sync_info=mybir.SyncInfo(
                                    on_wait=extra[i : i + max_waits], on_update=[]
                                ),
                            )
                        )
                    inst.sync_info = mybir.SyncInfo(
                        on_wait=keep, on_update=list(si.on_update)
                    )
                    changed = True
                new.append(inst)
            if changed:
                try:
                    blk.instructions = new
                except Exception:
                    blk.instructions.clear()
                    blk.instructions.extend(new)


def _fuse_ldw(nc):
    """Delete standalone InstLdweights; the following InstMatmult still
    carries the weights AP, so walrus emits it self-loading. Moves the LDW's
    waits/updates onto the matmul."""
    n = 0
    for fn in nc.m.functions:
        for blk in fn.blocks:
            insts = list(blk.instructions)
            new = []
            pending = None
            for inst in insts:
                if type(inst).__name__ == "InstLdweights":
                    pending = inst
                    continue
                if pending is not None:
                    psi = pending.sync_info
                    if psi is not None and (psi.on_wait or psi.on_update):
                        si = inst.sync_info
                        ow = list(psi.on_wait) + (list(si.on_wait) if si else [])
                        ou = (list(si.on_update) if si else []) + list(psi.on_update)
                        inst.sync_info = mybir.SyncInfo(on_wait=ow, on_update=ou)
                    pending = None
                    n += 1
                new.append(inst)
            if n:
                try:
                    blk.instructions = new
                except Exception:
                    blk.instructions.clear()
                    blk.instructions.extend(new)
    return n


def _dedupe_ldw(nc):
    """Remove back-to-back InstLdweights (per engine stream) that reload the
    exact same weights AP: the PE array keeps the stationary operand between
    matmuls, so a reload is pure overhead. Waits move to the next instruction."""
    import json as _json

    def key(inst):
        a = inst.ins[0]
        return (a.memref, a.offset, str(a.ap), str(a.dtype))

    n = 0
    for fn in nc.m.functions:
        for blk in fn.blocks:
            insts = list(blk.instructions)
            last_w = {}
            drop = set()
            pend_waits = {}
            new = []
            for inst in insts:
                eng = getattr(inst, "engine", None)
                nm = type(inst).__name__
                if nm == "InstLdweights":
                    k = key(inst)
                    if last_w.get(str(eng)) == k:
                        si = inst.sync_info
                        if si is not None and (si.on_wait or si.on_update):
                            pend_waits.setdefault(str(eng), []).append(si)
                        n += 1
                        continue
                    last_w[str(eng)] = k
                elif nm == "InstMatmult":
                    pw = pend_waits.pop(str(eng), None)
                    if pw:
                        si = inst.sync_info
                        ow = [w for p in pw for w in p.on_wait] + (
                            list(si.on_wait) if si else []
                        )
                        ou = (list(si.on_update) if si else []) + [
                            u for p in pw for u in p.on_update
                        ]
                        inst.sync_info = mybir.SyncInfo(on_wait=ow, on_update=ou)
                new.append(inst)
            if n:
                try:
                    blk.instructions = new
                except Exception:
                    blk.instructions.clear()
                    blk.instructions.extend(new)
    return n


def build_nc(split=True, stage=99):
    nc = bass.Bass()
    enc = nc.dram_tensor("enc", [S, BC, H], F32, kind="ExternalInput")
    hid = nc.dram_tensor("hid", [BC, H], F32, kind="ExternalInput")
    w_attn = nc.dram_tensor("w_attn", [H, 2 * H], F32, kind="ExternalInput")
    b_attn = nc.dram_tensor("b_attn", [H], F32, kind="ExternalInput")
    beta = nc.dram_tensor("beta", [H, 1], F32, kind="ExternalInput")
    out = nc.dram_tensor("out", [S, BC], F32, kind="ExternalOutput")

    with tile.TileContext(nc) as tc:
        _body(tc, enc, hid, w_attn, b_attn, beta, out, stage=stage)
    if os.environ.get("BASS_FUSE_LDW", "0") == "1":
        _fuse_ldw(nc)
    if os.environ.get("BASS_DEDUPE_LDW", "1") == "1":
        _dedupe_ldw(nc)
    if split:
        _split_waits(nc, max_waits=1)
    return nc


def _body(tc, enc, hid, w_attn, b_attn, beta, out, stage=99):
    nc = tc.nc
    import contextlib

    with contextlib.ExitStack() as ctx:
        const = ctx.enter_context(tc.tile_pool(name="const", bufs=1))
        epool = ctx.enter_context(tc.tile_pool(name="epool", bufs=4))
        etp = ctx.enter_context(tc.tile_pool(name="etp", bufs=1))
        thp = ctx.enter_context(tc.tile_pool(name="thp", bufs=3))
        pst = ctx.enter_context(tc.tile_pool(name="pst", bufs=2, space="PSUM"))
        pse = ctx.enter_context(tc.tile_pool(name="pse", bufs=4, space="PSUM"))
        psc = ctx.enter_context(tc.tile_pool(name="psc", bufs=2, space="PSUM"))

        Tanh = mybir.ActivationFunctionType.Tanh
        Exp = mybir.ActivationFunctionType.Exp

        # ---------------- PE warmup ----------------
        # Keep TensorE busy from t~1us so the HAM clock gate flips to
        # 2.4 GHz before the real matmul phase (needs ~3.4us sustained).
        identw = const.tile([128, 128], F32)
        make_identity(nc, identw)
        wps = pse.tile([128, SG], F32, tag="pe", name="wps")
        for _ in range(26):
            nc.tensor.transpose(wps[:, :128], identw, identw)

        ident16 = const.tile([128, 128], F16)
        make_identity(nc, ident16)
        ident8 = const.tile([BC, BC], F32)
        make_identity(nc, ident8)

        # small SWDGE loads (cast during DMA)
        betat = const.tile([128, KC], F16)
        with nc.allow_non_contiguous_dma(reason="512-element strided constant load"):
            nc.gpsimd.dma_start(
                out=betat, in_=beta.rearrange("(c p) o -> p (c o)", p=128)
            )
        hid16 = const.tile([BC, H], F16)
        nc.gpsimd.dma_start(out=hid16, in_=hid[:, :])
        batt = const.tile([128, HC], F32)
        with nc.allow_non_contiguous_dma(reason="512-element strided constant load"):
            nc.sync.dma_start(out=batt, in_=b_attn.rearrange("(c p) -> p c", p=128))

        # transposed E tiles: et[(b, sg)][k, kc, s] = E[sg*512+s, b, kc*128+k]
        et = {}
        for b in range(BC):
            for sg in range(SGS):
                et[(b, sg)] = etp.tile(
                    [128, KC, SG], F16, tag=f"et{b}_{sg}", name=f"et{b}_{sg}"
                )

        # W: cast to fp16 during DMA, transpose halves on PE.
        wt16 = const.tile([128, HC, 2 * H], F16)
        w1t = const.tile([128, KC, H], F16)
        w2t = const.tile([128, KC, H], F16)

        wt32 = const.tile([128, HC, 2 * H], F32)

        def load_w():
            nc.sync.dma_start(
                out=wt32, in_=w_attn.rearrange("(ho p) k -> p ho k", p=128)
            )
            nc.vector.tensor_copy(out=wt16, in_=wt32)

        def setup_w():
            for half, dst in ((0, w1t), (1, w2t)):
                for ho in range(HC):
                    ps = pst.tile([128, KC * 128], F16, tag="tr")
                    for kc in range(KC):
                        nc.tensor.transpose(
                            ps[:, kc * 128 : (kc + 1) * 128],
                            wt16[
                                :, ho, half * H + kc * 128 : half * H + (kc + 1) * 128
                            ],
                            ident16,
                        )
                    nc.vector.tensor_copy(
                        out=dst[:, :, ho * 128 : (ho + 1) * 128],
                        in_=ps.rearrange("p (kc h) -> p kc h", kc=KC),
                    )

        hidt = const.tile([128, KC, BC], F16)
        u_sb = const.tile([128, HC, BC], F32)

        def setup_u():
            for kc in range(KC):
                ps = pst.tile([128, BC], F16, tag="tr")
                nc.tensor.transpose(
                    ps, hid16[:, kc * 128 : (kc + 1) * 128], ident16[:BC, :BC]
                )
                nc.vector.tensor_copy(out=hidt[:, kc, :], in_=ps)
            # U[h, b] = W1[h, :] . hidden[b, :] + b_attn[h]
            for hc in range(HC):
                psu = psc.tile([128, BC], F32, tag="sc")
                for kc in range(KC):
                    nc.tensor.matmul(
                        psu,
                        w1t[:, kc, hc * 128 : (hc + 1) * 128],
                        hidt[:, kc, :],
                        start=(kc == 0),
                        stop=(kc == KC - 1),
                    )
                nc.vector.tensor_scalar_add(u_sb[:, hc, :], psu, batt[:, hc : hc + 1])

        # beta selector matrices: bsel[k, b, hc, col] = beta[hc*128+k] iff col==b
        bsel = const.tile([128, BC, KC, BC], F16)
        nc.vector.memset(bsel, 0.0)
        for b in range(BC):
            for hc in range(HC):
                nc.vector.tensor_copy(
                    out=bsel[:, b, hc, b : b + 1], in_=betat[:, hc : hc + 1]
                )

        scores = const.tile([BC, S], F32)
        mx2 = const.tile([BC, SGS], F32)

        def load_chunk(sg, j, sj):
            # one [128 s, 2 b, 512 h] chunk, fp32->fp16 cast in the DMA
            si = sg * (SI // SGS) + sj
            c16 = epool.tile([128, 2, H], F16, tag="e16")
            nc.gpsimd.dma_start(
                out=c16, in_=enc[si * 128 : (si + 1) * 128, 2 * j : 2 * j + 2, :]
            )
            # PE-transpose the 8 [128,128] blocks via one PSUM staging tile
            ps = pst.tile([128, 2 * KC * 128], F16, tag="tr")
            for bb in range(2):
                for kc in range(KC):
                    nc.tensor.transpose(
                        ps[:, (bb * KC + kc) * 128 : (bb * KC + kc + 1) * 128],
                        c16[:, bb, kc * 128 : (kc + 1) * 128],
                        ident16,
                    )
            for bb in range(2):
                nc.vector.tensor_copy(
                    out=et[(2 * j + bb, sg)][:, :, sj * 128 : (sj + 1) * 128],
                    in_=ps[:, bb * KC * 128 : (bb + 1) * KC * 128].rearrange(
                        "p (kc s) -> p kc s", kc=KC
                    ),
                )

        def mains(b, sg, th):
            for hc in range(HC):
                pe = pse.tile([128, SG], F32, tag="pe", name=f"pe{b % 2}")
                for kc in range(KC):
                    nc.tensor.matmul(
                        pe,
                        w2t[:, kc, hc * 128 : (hc + 1) * 128],
                        et[(b, sg)][:, kc, :],
                        start=(kc == 0),
                        stop=(kc == KC - 1),
                    )
                # tanh(energy + U[:, b]) fused on ScalarE, fp16 out
                nc.scalar.activation(
                    out=th[:, hc, :],
                    in_=pe,
                    func=Tanh,
                    bias=u_sb[:, hc, b : b + 1],
                    scale=1.0,
                )

        def beta_mms(b, sg, th, pss):
            for hc in range(HC):
                nc.tensor.matmul(
                    pss,
                    bsel[:, b, hc, :],
                    th[:, hc, :],
                    start=(b == 0 and hc == 0),
                    stop=(b == BC - 1 and hc == HC - 1),
                )

        # ---------------- main pipeline ----------------
        # Loads ordered (sg, b-pair, s-chunk); pair j's transposes interleave
        # with pair j-1's main matmuls; beta matmuls trail one b behind.
        load_w()
        setup_w()
        setup_u()

        pss = {}
        for sg in range(SGS):
            pss[sg] = psc.tile([BC, SG], F32, tag="sc", name=f"pss{sg}")
            ths = {}

            def run_b(b, sg=sg):
                ths[b] = thp.tile([128, HC, SG], F16, tag="th", name=f"th{sg}_{b}")
                mains(b, sg, ths[b])
                if b > 0:
                    beta_mms(b - 1, sg, ths[b - 1], pss[sg])

            for j in range(BC // 2):
                for sj in range(SI // SGS):
                    load_chunk(sg, j, sj)
                if j >= 1:
                    run_b(2 * (j - 1))
                    run_b(2 * (j - 1) + 1)
            run_b(BC - 2)
            run_b(BC - 1)
            beta_mms(BC - 1, sg, ths[BC - 1], pss[sg])
            nc.vector.tensor_copy(out=scores[:, sg * SG : (sg + 1) * SG], in_=pss[sg])
            nc.vector.reduce_max(
                mx2[:, sg : sg + 1],
                scores[:, sg * SG : (sg + 1) * SG],
                axis=mybir.AxisListType.X,
            )

        # ---------------- softmax over s (free dim), per b (partition) ----
        osb = const.tile([128, SI, BC], F32)
        nmx = const.tile([BC, 1], F32)
        nc.vector.reduce_max(nmx, mx2, axis=mybir.AxisListType.X, negate=True)
        ex = const.tile([BC, S], F32)
        nc.scalar.activation(out=ex, in_=scores, func=Exp, bias=nmx, scale=1.0)
        sm = const.tile([BC, 1], F32)
        nc.vector.reduce_sum(sm, ex, axis=mybir.AxisListType.X)
        rp = const.tile([BC, 1], F32)
        nc.vector.reciprocal(rp, sm)
        # D = diag(1/sum): transpose-and-normalize in one PE op per chunk:
        # out[s, b] = sum_k ex[k, s] * D[k, b] = ex[b, s] / sum_b
        dmat = const.tile([BC, BC], F32)
        nc.vector.tensor_scalar_mul(dmat, ident8, rp)
        for si in range(SI):
            po = psc.tile([128, BC], F32, tag="sc")
            nc.tensor.matmul(
                po, ex[:, si * 128 : (si + 1) * 128], dmat, start=True, stop=True
            )
            nc.vector.tensor_copy(out=osb[:, si, :], in_=po)
        nc.sync.dma_start(
            out=out.rearrange("(si p) b -> p si b", p=128), in_=osb
        )


_NC_CACHE = None


def _get_nc():
    global _NC_CACHE
    if _NC_CACHE is None:
        _NC_CACHE = build_nc()
    return _NC_CACHE


def run(inputs, trace=False, **kw):
    """Shard, execute on 8 NeuronCores, gather. Returns (output, BassKernelResults)."""
    hidden = np.asarray(inputs["hidden"], dtype=np.float32)
    enc = np.ascontiguousarray(np.asarray(inputs["encoder_outputs"], dtype=np.float32))
    w_attn = np.ascontiguousarray(np.asarray(inputs["W_attn"], dtype=np.float32))
    b_attn = np.ascontiguousarray(np.asarray(inputs["b_attn"], dtype=np.float32))
    beta = np.ascontiguousarray(np.asarray(inputs["beta"], dtype=np.float32))

    nc = _get_nc()
    in_maps = []
    for c in range(NCORES):
        b0 = c * BC
        in_maps.append(
            {
                "enc": np.ascontiguousarray(enc[:, b0 : b0 + BC, :]),
                "hid": np.ascontiguousarray(hidden[0, b0 : b0 + BC, :]),
                "w_attn": w_attn,
                "b_attn": b_attn,
                "beta": beta,
            }
        )
    res = run_bass_kernel_spmd(
        nc, in_maps, core_ids=list(range(NCORES)), trace=trace, **kw
    )
    outs = [res.results[c]["out"] for c in range(NCORES)]  # each [S, BC]
    full = np.concatenate(outs, axis=1)  # [S, B]
    return full[:, None, :].astype(np.float32), res  # [S, 1, B]


def kernel(**inputs):
    out, _ = run(inputs, trace=False)
    return out


# revision 27
# speedup vs baseline: 1.0949x; 1.0949x over previous
"""Bass/Trainium2 kernel for nn_Attn_51127290691658.

Reference computation (S=1024, B=64, H=512):
    cat    = concat([broadcast(hidden), encoder_outputs], -1)   [S,B,2H]
    energy = tanh(cat @ W_attn.T + b_attn)                      [S,B,H]
    scores = energy @ beta                                      [S,B,1]
    out    = softmax(scores.transpose(0,2,1), axis=0)           [S,1,B]

Decomposition used here (W1 = W_attn[:, :H], W2 = W_attn[:, H:]):
    U[b,h]      = W1[h,:] . hidden[b,:] + b_attn[h]        (tiny)
    energyT[h,s] = tanh(W2 @ E_b^T + U[:,b])   per batch b (big)
    score[b,s]  = beta . tanh_energy[:, s]
    out[s,b]    = softmax over s

Sharding: data-parallel over B across 8 cores (8 batch elements/core);
W_attn/b_attn/beta replicated. Softmax is local per batch element.

Data path: fp16 inputs to the PE matmuls (full-rate 1 cyc/row on TRN2),
fp32 PSUM accumulation, fp32 softmax. Measured end-to-end absmax error
vs the fp32 reference ~3.5e-3 (softmax output scale is 1.0).
"""

import sys
import types

import numpy as np

S, B, H = 1024, 64, 512
NCORES = 8
BC = B // NCORES  # 8 batch elements per core
KC = H // 128     # 4 contraction chunks
HC = H // 128     # 4 output h chunks
SGS = 2           # two 512-wide s groups
SG = S // SGS     # 512
SI = S // 128     # 8 s chunks of 128


def _install_axon_hooks_shim():
    """The container image's `antenv` lacks `axon_hooks`; without it,
    run_bass_kernel_spmd(trace=True) cannot find the NTFF hook. Register a
    minimal in-memory module and install the ctypes-based hook if available.
    Harmless when tracing is not requested."""
    try:
        import antenv
    except ImportError:
        return
    if "antenv.axon_hooks" in sys.modules:
        return
    mod = types.ModuleType("antenv.axon_hooks")
    mod._hook = None

    def set_axon_ntff_profile_hook(h):
        mod._hook = h

    def get_axon_ntff_profile_hook():
        return mod._hook

    mod.set_axon_ntff_profile_hook = set_axon_ntff_profile_hook
    mod.get_axon_ntff_profile_hook = get_axon_ntff_profile_hook
    sys.modules["antenv.axon_hooks"] = mod
    antenv.axon_hooks = mod
    try:
        from trn_agent_boot.trn_boot import _ntff_profile_via_ctypes

        hook = _ntff_profile_via_ctypes("/opt/axon/libaxon_pjrt.so")
        if hook is not None:
            set_axon_ntff_profile_hook(hook)
    except Exception:
        pass


_install_axon_hooks_shim()

import os  # noqa: E402

if os.environ.get("BASS_LDW_OPT") == "1":
    import concourse.bass_utils as _bu

    _orig_run_command = _bu.run_command

    def _patched_run_command(argv, **kw):
        argv = [
            a.replace("--enable-ldw-opt=false", "--enable-ldw-opt=true") for a in argv
        ]
        return _orig_run_command(argv, **kw)

    _bu.run_command = _patched_run_command

import concourse.bass as bass  # noqa: E402
import concourse.mybir as mybir  # noqa: E402
import concourse.tile as tile  # noqa: E402
from concourse.bass_utils import run_bass_kernel_spmd  # noqa: E402
from concourse.masks import make_identity  # noqa: E402

F32 = mybir.dt.float32
F16 = mybir.dt.float16


def _split_waits(nc, max_waits=1):
    """The walrus build in this container encodes at most one sem-wait per
    instruction ("Too many sync wait commands" otherwise). Tile emits up to
    ~5. Splitting excess waits into preceding same-engine NoOps is
    semantically identical (engine queues execute in order)."""
    ctr = 0
    for fn in nc.m.functions:
        for blk in fn.blocks:
            insts = list(blk.instructions)
            new = []
            changed = False
            for inst in insts:
                si = inst.sync_info
                if si is not None and len(si.on_wait) > max_waits:
                    waits = list(si.on_wait)
                    keep = waits[-max_waits:]
                    extra = waits[:-max_waits]
                    for i in range(0, len(extra), max_waits):
                        ctr += 1
                        new.append(
                            mybir.InstNoOp(
                                name=f"WSPLIT-{id(nc) & 0xFFFF}-{ctr}",
                                engine=inst.engine,
                                bass_nofuse=True,
                                sync_info=mybir.SyncInfo(
                                    on_wait=extra[i : i + max_waits], on_update=[]
                                ),
                            )
                        )
                    inst.sync_info = mybir.SyncInfo(
                        on_wait=keep, on_update=list(si.on_update)
                    )
                    changed = True
                new.append(inst)
            if changed:
                try:
                    blk.instructions = new
                except Exception:
                    blk.instructions.clear()
                    blk.instructions.extend(new)


def _fuse_ldw(nc):
    """Delete standalone InstLdweights; the following InstMatmult still
    carries the weights AP, so walrus emits it self-loading. Moves the LDW's
    waits/updates onto the matmul."""
    n = 0
    for fn in nc.m.functions:
        for blk in fn.blocks:
            insts = list(blk.instructions)
            new = []
            pending = None
            for inst in insts:
                if type(inst).__name__ == "InstLdweights":
                    pending = inst
                    continue
                if pending is not None:
                    psi = pending.sync_info
                    if psi is not None and (psi.on_wait or psi.on_update):
                        si = inst.sync_info
                        ow = list(psi.on_wait) + (list(si.on_wait) if si else [])
                        ou = (list(si.on_update) if si else []) + list(psi.on_update)
                        inst.sync_info = mybir.SyncInfo(on_wait=ow, on_update=ou)
                    pending = None
                    n += 1
                new.append(inst)
            if n:
                try:
                    blk.instructions = new
                except Exception:
                    blk.instructions.clear()
                    blk.instructions.extend(new)
    return n


def _dedupe_ldw(nc):
    """Remove back-to-back InstLdweights (per engine stream) that reload the
    exact same weights AP: the PE array keeps the stationary operand between
    matmuls, so a reload is pure overhead. Waits move to the next instruction."""
    import json as _json

    def key(inst):
        a = inst.ins[0]
        return (a.memref, a.offset, str(a.ap), str(a.dtype))

    n = 0
    for fn in nc.m.functions:
        for blk in fn.blocks:
            insts = list(blk.instructions)
            last_w = {}
            drop = set()
            pend_waits = {}
            new = []
            for inst in insts:
                eng = getattr(inst, "engine", None)
                nm = type(inst).__name__
                if nm == "InstLdweights":
                    k = key(inst)
                    if last_w.get(str(eng)) == k:
                        si = inst.sync_info
                        if si is not None and (si.on_wait or si.on_update):
                            pend_waits.setdefault(str(eng), []).append(si)
                        n += 1
                        continue
                    last_w[str(eng)] = k
                elif nm == "InstMatmult":
                    pw = pend_waits.pop(str(eng), None)
                    if pw:
                        si = inst.sync_info
                        ow = [w for p in pw for w in p.on_wait] + (
                            list(si.on_wait) if si else []
                        )
                        ou = (list(si.on_update) if si else []) + [
                            u for p in pw for u in p.on_update
                        ]
                        inst.sync_info = mybir.SyncInfo(on_wait=ow, on_update=ou)
                new.append(inst)
            if n:
                try:
                    blk.instructions = new
                except Exception:
                    blk.instructions.clear()
                    blk.instructions.extend(new)
    return n


def build_nc(split=True, stage=99):
    nc = bass.Bass()
    enc = nc.dram_tensor("enc", [S, BC, H], F32, kind="ExternalInput")
    hid = nc.dram_tensor("hid", [BC, H], F32, kind="ExternalInput")
    w_attn = nc.dram_tensor("w_attn", [H, 2 * H], F32, kind="ExternalInput")
    b_attn = nc.dram_tensor("b_attn", [H], F32, kind="ExternalInput")
    beta = nc.dram_tensor("beta", [H, 1], F32, kind="ExternalInput")
    out = nc.dram_tensor("out", [S, BC], F32, kind="ExternalOutput")

    with tile.TileContext(nc) as tc:
        _body(tc, enc, hid, w_attn, b_attn, beta, out, stage=stage)
    if os.environ.get("BASS_FUSE_LDW", "0") == "1":
        _fuse_ldw(nc)
    if os.environ.get("BASS_DEDUPE_LDW", "1") == "1":
        _dedupe_ldw(nc)
    if split:
        _split_waits(nc, max_waits=1)
    return nc


def _body(tc, enc, hid, w_attn, b_attn, beta, out, stage=99):
    nc = tc.nc
    import contextlib

    with contextlib.ExitStack() as ctx:
        const = ctx.enter_context(tc.tile_pool(name="const", bufs=1))
        epool = ctx.enter_context(tc.tile_pool(name="epool", bufs=4))
        etp = ctx.enter_context(tc.tile_pool(name="etp", bufs=1))
        thp = ctx.enter_context(tc.tile_pool(name="thp", bufs=3))
        pst = ctx.enter_context(tc.tile_pool(name="pst", bufs=2, space="PSUM"))
        pse = ctx.enter_context(tc.tile_pool(name="pse", bufs=4, space="PSUM"))
        psc = ctx.enter_context(tc.tile_pool(name="psc", bufs=2, space="PSUM"))

        Tanh = mybir.ActivationFunctionType.Tanh
        Exp = mybir.ActivationFunctionType.Exp

        # ---------------- PE warmup ----------------
        # Keep TensorE busy from t~1us so the HAM clock gate flips to
        # 2.4 GHz before the real matmul phase (needs ~3.4us sustained).
        identw = const.tile([128, 128], F32)
        make_identity(nc, identw)
        wps = pse.tile([128, SG], F32, tag="pe", name="wps")
        for _ in range(22):
            nc.tensor.transpose(wps[:, :128], identw, identw)

        ident16 = const.tile([128, 128], F16)
        make_identity(nc, ident16)
        ident8 = const.tile([BC, BC], F32)
        make_identity(nc, ident8)

        # small SWDGE loads (cast during DMA)
        betat = const.tile([128, KC], F16)
        with nc.allow_non_contiguous_dma(reason="512-element strided constant load"):
            nc.gpsimd.dma_start(
                out=betat, in_=beta.rearrange("(c p) o -> p (c o)", p=128)
            )
        hid16 = const.tile([BC, H], F16)
        nc.gpsimd.dma_start(out=hid16, in_=hid[:, :])
        batt = const.tile([128, HC], F32)
        with nc.allow_non_contiguous_dma(reason="512-element strided constant load"):
            nc.sync.dma_start(out=batt, in_=b_attn.rearrange("(c p) -> p c", p=128))

        # transposed E tiles: et[(b, sg)][k, kc, s] = E[sg*512+s, b, kc*128+k]
        et = {}
        for b in range(BC):
            for sg in range(SGS):
                et[(b, sg)] = etp.tile(
                    [128, KC, SG], F16, tag=f"et{b}_{sg}", name=f"et{b}_{sg}"
                )

        # W: cast to fp16 during DMA, transpose halves on PE.
        wt16 = const.tile([128, HC, 2 * H], F16)
        w1t = const.tile([128, KC, H], F16)
        w2t = const.tile([128, KC, H], F16)

        def load_w():
            nc.gpsimd.dma_start(
                out=wt16, in_=w_attn.rearrange("(ho p) k -> p ho k", p=128)
            )

        def setup_w():
            for half, dst in ((0, w1t), (1, w2t)):
                for ho in range(HC):
                    ps = pst.tile([128, KC * 128], F16, tag="tr")
                    for kc in range(KC):
                        nc.tensor.transpose(
                            ps[:, kc * 128 : (kc + 1) * 128],
                            wt16[
                                :, ho, half * H + kc * 128 : half * H + (kc + 1) * 128
                            ],
                            ident16,
                        )
                    nc.vector.tensor_copy(
                        out=dst[:, :, ho * 128 : (ho + 1) * 128],
                        in_=ps.rearrange("p (kc h) -> p kc h", kc=KC),
                    )

        hidt = const.tile([128, KC, BC], F16)
        u_sb = const.tile([128, HC, BC], F32)

        def setup_u():
            for kc in range(KC):
                ps = pst.tile([128, BC], F16, tag="tr")
                nc.tensor.transpose(
                    ps, hid16[:, kc * 128 : (kc + 1) * 128], ident16[:BC, :BC]
                )
                nc.vector.tensor_copy(out=hidt[:, kc, :], in_=ps)
            # U[h, b] = W1[h, :] . hidden[b, :] + b_attn[h]
            for hc in range(HC):
                psu = psc.tile([128, BC], F32, tag="sc")
                for kc in range(KC):
                    nc.tensor.matmul(
                        psu,
                        w1t[:, kc, hc * 128 : (hc + 1) * 128],
                        hidt[:, kc, :],
                        start=(kc == 0),
                        stop=(kc == KC - 1),
                    )
                nc.vector.tensor_scalar_add(u_sb[:, hc, :], psu, batt[:, hc : hc + 1])

        # beta selector matrices: bsel[k, b, hc, col] = beta[hc*128+k] iff col==b
        bsel = const.tile([128, BC, KC, BC], F16)
        nc.vector.memset(bsel, 0.0)
        for b in range(BC):
            for hc in range(HC):
                nc.vector.tensor_copy(
                    out=bsel[:, b, hc, b : b + 1], in_=betat[:, hc : hc + 1]
                )

        mx2 = const.tile([BC, SGS], F32)
        e_sb = const.tile([BC, SGS, SG], F32)
        t0 = const.tile([BC, 1], F32)

        def load_chunk(sg, j, sj):
            # one [128 s, 2 b, 512 h] chunk, fp32->fp16 cast in the DMA
            si = sg * (SI // SGS) + sj
            c16 = epool.tile([128, 2, H], F16, tag="e16")
            nc.gpsimd.dma_start(
                out=c16, in_=enc[si * 128 : (si + 1) * 128, 2 * j : 2 * j + 2, :]
            )
            # PE-transpose the 8 [128,128] blocks via one PSUM staging tile
            ps = pst.tile([128, 2 * KC * 128], F16, tag="tr")
            for bb in range(2):
                for kc in range(KC):
                    nc.tensor.transpose(
                        ps[:, (bb * KC + kc) * 128 : (bb * KC + kc + 1) * 128],
                        c16[:, bb, kc * 128 : (kc + 1) * 128],
                        ident16,
                    )
            for bb in range(2):
                nc.vector.tensor_copy(
                    out=et[(2 * j + bb, sg)][:, :, sj * 128 : (sj + 1) * 128],
                    in_=ps[:, bb * KC * 128 : (bb + 1) * KC * 128].rearrange(
                        "p (kc s) -> p kc s", kc=KC
                    ),
                )

        def mains(b, sg, th):
            for hc in range(HC):
                pe = pse.tile([128, SG], F32, tag="pe", name=f"pe{b % 2}")
                for kc in range(KC):
                    nc.tensor.matmul(
                        pe,
                        w2t[:, kc, hc * 128 : (hc + 1) * 128],
                        et[(b, sg)][:, kc, :],
                        start=(kc == 0),
                        stop=(kc == KC - 1),
                    )
                # tanh(energy + U[:, b]) fused on ScalarE, fp16 out
                nc.scalar.activation(
                    out=th[:, hc, :],
                    in_=pe,
                    func=Tanh,
                    bias=u_sb[:, hc, b : b + 1],
                    scale=1.0,
                )

        def beta_mms(b, sg, th, pss):
            for hc in range(HC):
                nc.tensor.matmul(
                    pss,
                    bsel[:, b, hc, :],
                    th[:, hc, :],
                    start=(b == 0 and hc == 0),
                    stop=(b == BC - 1 and hc == HC - 1),
                )

        # ---------------- main pipeline ----------------
        # Loads ordered (sg, b-pair, s-chunk); pair j's transposes interleave
        # with pair j-1's main matmuls; beta matmuls trail one b behind.
        load_w()
        setup_w()
        setup_u()

        pss = {}
        for sg in range(SGS):
            pss[sg] = psc.tile([BC, SG], F32, tag="sc", name=f"pss{sg}")
            ths = {}

            def run_b(b, sg=sg):
                ths[b] = thp.tile([128, HC, SG], F16, tag="th", name=f"th{sg}_{b}")
                mains(b, sg, ths[b])
                if b > 0:
                    beta_mms(b - 1, sg, ths[b - 1], pss[sg])

            for j in range(BC // 2):
                for sj in range(SI // SGS):
                    load_chunk(sg, j, sj)
                if j >= 1:
                    run_b(2 * (j - 1))
                    run_b(2 * (j - 1) + 1)
            run_b(BC - 2)
            run_b(BC - 1)
            beta_mms(BC - 1, sg, ths[BC - 1], pss[sg])
            nc.vector.reduce_max(
                mx2[:, sg : sg + 1], pss[sg], axis=mybir.AxisListType.X
            )
            if sg == 0:
                # online softmax: exp/sum of the first half against its own
                # max, rescaled later; hides this work under sg1's matmuls
                nmx0 = const.tile([BC, 1], F32)
                nc.vector.tensor_scalar_mul(nmx0, mx2[:, 0:1], -1.0)
                nc.scalar.activation(
                    out=e_sb[:, 0, :], in_=pss[0], func=Exp, bias=nmx0, scale=1.0
                )
                nc.vector.reduce_sum(t0, e_sb[:, 0, :], axis=mybir.AxisListType.X)

        # ---------------- softmax tail (second half + rescale) ----------
        osb = const.tile([128, SI, BC], F32)
        nmx = const.tile([BC, 1], F32)
        nc.vector.reduce_max(nmx, mx2, axis=mybir.AxisListType.X, negate=True)
        nc.scalar.activation(
            out=e_sb[:, 1, :], in_=pss[1], func=Exp, bias=nmx, scale=1.0
        )
        # c0 = exp(mx0 - M) rescales the first half's exp/sum
        c0 = const.tile([BC, 1], F32)
        nc.scalar.activation(out=c0, in_=mx2[:, 0:1], func=Exp, bias=nmx, scale=1.0)
        sm = const.tile([BC, 1], F32)
        nc.vector.reduce_sum(sm, e_sb[:, 1, :], axis=mybir.AxisListType.X)
        t0c = const.tile([BC, 1], F32)
        nc.vector.tensor_mul(t0c, t0, c0)
        nc.vector.tensor_add(sm, sm, t0c)
        rp = const.tile([BC, 1], F32)
        nc.vector.reciprocal(rp, sm)
        rpc = const.tile([BC, 1], F32)
        nc.vector.tensor_mul(rpc, rp, c0)
        # D = diag(scale): transpose-and-normalize in one PE op per chunk:
        # out[s, b] = sum_k e[k, s] * D[k, b] = e[b, s] * scale_b
        dmat0 = const.tile([BC, BC], F32)
        nc.vector.tensor_scalar_mul(dmat0, ident8, rpc)
        dmat1 = const.tile([BC, BC], F32)
        nc.vector.tensor_scalar_mul(dmat1, ident8, rp)
        for si in range(SI):
            sg = si // (SI // SGS)
            sj = si % (SI // SGS)
            po = psc.tile([128, BC], F32, tag="sc")
            nc.tensor.matmul(
                po,
                e_sb[:, sg, sj * 128 : (sj + 1) * 128],
                dmat0 if sg == 0 else dmat1,
                start=True,
                stop=True,
            )
            nc.vector.tensor_copy(out=osb[:, si, :], in_=po)
        nc.sync.dma_start(
            out=out.rearrange("(si p) b -> p si b", p=128), in_=osb
        )


_NC_CACHE = None


def _get_nc():
    global _NC_CACHE
    if _NC_CACHE is None:
        _NC_CACHE = build_nc()
    return _NC_CACHE


def run(inputs, trace=False, **kw):
    """Shard, execute on 8 NeuronCores, gather. Returns (output, BassKernelResults)."""
    hidden = np.asarray(inputs["hidden"], dtype=np.float32)
    enc = np.ascontiguousarray(np.asarray(inputs["encoder_outputs"], dtype=np.float32))
    w_attn = np.ascontiguousarray(np.asarray(inputs["W_attn"], dtype=np.float32))
    b_attn = np.ascontiguousarray(np.asarray(inputs["b_attn"], dtype=np.float32))
    beta = np.ascontiguousarray(np.asarray(inputs["beta"], dtype=np.float32))

    nc = _get_nc()
    in_maps = []
    for c in range(NCORES):
        b0 = c * BC
        in_maps.append(
            {
                "enc": np.ascontiguousarray(enc[:, b0 : b0 + BC, :]),
                "hid": np.ascontiguousarray(hidden[0, b0 : b0 + BC, :]),
                "w_attn": w_attn,
                "b_attn": b_attn,
                "beta": beta,
            }
        )
    res = run_bass_kernel_spmd(
        nc, in_maps, core_ids=list(range(NCORES)), trace=trace, **kw
    )
    outs = [res.results[c]["out"] for c in range(NCORES)]  # each [S, BC]
    full = np.concatenate(outs, axis=1)  # [S, B]
    return full[:, None, :].astype(np.float32), res  # [S, 1, B]


def kernel(**inputs):
    out, _ = run(inputs, trace=False)
    return out
